# revision 13
# baseline (speedup 1.0000x reference)
"""NT-Xent / contrastive loss on 8 Trainium2 NeuronCores (all-gather design).

Reference computation (B=4096, D=512, temp=0.1):
    z   = l2norm(concat(proj_1, proj_2))          # [8192, 512]
    cos = (z @ z.T) / temp                        # [8192, 8192]
    pos[r]  = cos[r, (r + 4096) % 8192]
    lse[r]  = logsumexp(cos[r, :] with cos[r, r] masked out)
    loss    = mean(lse - pos)

Sharding: CYCLIC rows — core c owns global rows {8*i + c}.  This keeps
(r, r+4096) positive pairs on one core (local rows i and i+512) and makes
the program SPMD-uniform.  Each core receives only its own 1024 rows.

Per core:
  1. Cast-DMA its 8 row-tiles in as bf16; row sumsq (DVE STT);
     1/||row|| via fast-rsqrt + 2 Newton steps (DVE).
  2. Normalize folded into the PE transpose (moving operand diag(rn*S));
     evacuate PSUM to an fp8 strip zq [128, 4k, 1024] (DVE).
  3. AllGather the fp8 strips via DRAM bounce buffers (2 pipelined halves,
     collective runs on TOPSP/SDMA silicon), read back into
     zt [128, 4k, 8192] in 4 chunks.  Column order is a permutation of
     global rows; row sums are permutation-invariant.
  4. GEMM rows x all columns in fp8 DoubleRow (2x PE rate): 8 row-blocks
     x 4 column groups of 2048.  Per group the exp+row-sum runs on:
       - ACT (table Exp with accum_out) for row blocks 0-5,
       - DVE (Schraudolph bit-trick exp + reduce) for blocks 6-7,
     splitting the exp wall across two engines.
  5. Self/pos logits are computed from the LOCAL strip (exact same fp8
     products the GEMM used): prod = zq*zq (bf16), ones-matmuls give
     self[i]=|zq_i|^2 and pos[i]=zq_i.zq_{i+512} in [128, block] layout.
     lse = ln(rowsum - expE(self)) with expE matching the block's engine.
  6. partial = sum(lse) - 2*SCL*sum(pos); host adds partials / 8192.
"""

import sys

import numpy as np

if "/opt/trn_rl_repo" not in sys.path:
    sys.path.insert(0, "/opt/trn_rl_repo")

_B = 4096
_D = 512
_N2 = 2 * _B            # 8192 rows of the similarity matrix
_NCORES = 8
_RPC = _N2 // _NCORES   # 1024 rows per core
_INV_TEMP = 10.0
_S = 64.0               # fp8 pre-scale on normalized rows
_SCL = _INV_TEMP / (_S * _S)   # logit scale applied at exp time

_NT = _RPC // 128       # 8 local row tiles
_NK = _D // 128         # 4 contraction chunks (2 DoubleRow pairs)
_NM = _RPC // 128       # 8 output row blocks per core
_CGRP = (1024, 1024, 1536, 1536, 1536, 1536)   # GEMM column-group widths
_NG = len(_CGRP)
_RBW = 2048             # readback chunk width (4 chunks)
_DVE_BLOCKS = (6, 7)    # row blocks whose exp runs on DVE (Schraudolph)
_MORDER = (0, 6, 1, 2, 7, 3, 4, 5)   # interleave DVE blocks for overlap

_MAGIC1 = 0x5F3759E0    # fast inverse sqrt magic + 1 (M - x == (M+1) + ~x)

# Schraudolph exp: exp(SCL*p) ~= bitcast_f32(int(A*p + Bc))
_SCHR_A = _SCL * (2.0 ** 23) / float(np.log(2.0))
_SCHR_B = 127.0 * 2.0 ** 23 - 486411.0   # ~zero-mean error constant


def _emit(tc, projs, out_partial):
    import concourse.bass as bass  # noqa: F401
    from concourse import mybir

    nc = tc.nc
    f32 = mybir.dt.float32
    bf16 = mybir.dt.bfloat16
    fp8 = mybir.dt.float8e4
    i32 = mybir.dt.int32
    Alu = mybir.AluOpType
    Act = mybir.ActivationFunctionType
    DR = mybir.MatmulPerfMode.DoubleRow

    from contextlib import ExitStack
    ctx = ExitStack()
    pool = ctx.enter_context(tc.tile_pool(name="work", bufs=1))
    pers = ctx.enter_context(tc.tile_pool(name="pers", bufs=1))
    pspool = ctx.enter_context(tc.tile_pool(name="psum", bufs=1, space="PSUM"))
    dram = ctx.enter_context(tc.tile_pool(name="dram", bufs=1, space="DRAM"))

    # ---- input stream first: 8 cast-DMAs on the gpsimd SWDGE queue ----
    raw_all = pers.tile([128, _NT * _D], bf16, tag="raw")
    raw3 = raw_all.rearrange("p (t d) -> p t d", t=_NT)
    projs4 = projs.rearrange("(h t p) d -> h p t d", p=128, t=_NT // 2)
    for h in range(2):
        nc.gpsimd.dma_start(raw3[:, 4 * h:4 * h + 4, :], projs4[h])

    # ---- constants (off the gpsimd queue where possible) ----
    ones = pers.tile([128, 128], f32, tag="ones")
    nc.vector.memset(ones[:], 1.0)
    ident = pers.tile([128, 128], f32, tag="ident")
    nc.gpsimd.affine_select(ident[:], ones[:], pattern=[[1, 128]],
                            compare_op=Alu.is_equal, fill=0.0,
                            base=0, channel_multiplier=-1)
    identb = pers.tile([128, 128], bf16, tag="identb")
    nc.vector.tensor_copy(identb[:], ident[:])

    # ---- persistent buffers ----
    zq = pers.tile([128, _NK * _RPC], fp8, tag="zq")      # own strip
    zq3 = zq.rearrange("p (k c) -> p k c", k=_NK)
    zt = pers.tile([128, _NK * _N2], fp8, tag="zt")       # gathered
    zt3 = zt.rearrange("p (k c) -> p k c", k=_NK)
    se_all = pers.tile([128, _NM * _NG], f32, tag="se")   # group sums
    se3 = se_all.rearrange("p (m g) -> p m g", m=_NM)
    rs_all = pers.tile([128, _NM], f32, tag="rs")         # row sumexp
    spv = pers.tile([128, 16], f32, tag="spv")            # self[0:8] pos[8:12]
    sx = pers.tile([128, _NM], f32, tag="sx")             # exp(self) per block

    # ---- phase 1+2 per half: sumsq -> rsqrt -> transpose/evac -> AG ----
    # Strip half h covers local cols w in [512h, 512h+512) across all k.
    # zt column layout: j = 4096*h + 512*s + w'  (s = source core).
    HW = _RPC // 2   # 512 cols per half
    ss = pool.tile([128, _NT], f32, tag="ss")
    gath = []
    for h in range(2):
        for t in range(4 * h, 4 * h + 4):
            sq = pool.tile([128, _D], bf16, tag="sq", bufs=2, name=f"sq{t}")
            nc.vector.scalar_tensor_tensor(
                out=sq[:], in0=raw3[:, t, :], scalar=1.0, in1=raw3[:, t, :],
                op0=Alu.mult, op1=Alu.mult, accum_out=ss[:, t:t + 1])

        # rnorm = S/sqrt(max(ss, 1e-24)), fast-rsqrt + 2 Newton steps (DVE)
        sh = ss[:, 4 * h:4 * h + 4]
        ssc = pool.tile([128, 4], f32, tag="ssc", bufs=2, name=f"ssc{h}")
        nc.vector.tensor_scalar_max(ssc[:], sh, 1e-24)
        ti = pool.tile([128, 4], i32, tag="ti", bufs=2, name=f"ti{h}")
        nc.vector.tensor_scalar(
            out=ti[:], in0=ssc[:].bitcast(i32), scalar1=1, scalar2=-1,
            op0=Alu.logical_shift_right, op1=Alu.bitwise_xor)
        rn = pool.tile([128, 4], f32, tag="rn", bufs=2, name=f"rn{h}")
        nc.vector.tensor_scalar(
            out=rn[:].bitcast(i32), in0=ti[:], scalar1=_MAGIC1, scalar2=None,
            op0=Alu.add)
        nt = pool.tile([128, 4], f32, tag="nt", bufs=2, name=f"nt{h}")
        for _ in range(2):
            nc.vector.tensor_tensor(out=nt[:], in0=rn[:], in1=rn[:],
                                    op=Alu.mult)
            nc.vector.tensor_tensor(out=nt[:], in0=nt[:], in1=ssc[:],
                                    op=Alu.mult)
            nc.vector.tensor_scalar(out=nt[:], in0=nt[:], scalar1=-0.5,
                                    scalar2=1.5, op0=Alu.mult, op1=Alu.add)
            nc.vector.tensor_tensor(out=rn[:], in0=rn[:], in1=nt[:],
                                    op=Alu.mult)
        rnsc = pool.tile([128, 4], f32, tag="rnsc", bufs=2, name=f"rnsc{h}")
        nc.vector.tensor_scalar_mul(rnsc[:], rn[:], _S)

        for i, t in enumerate(range(4 * h, 4 * h + 4)):
            diag = pool.tile([128, 128], bf16, tag="diag", bufs=2,
                             name=f"dg{t}")
            nc.vector.tensor_scalar_mul(diag[:], identb[:], rnsc[:, i:i + 1])
            psT = pspool.tile([128, _D], f32, tag="pst", bufs=2,
                              name=f"psT{t}")
            for d in range(_NK):
                nc.tensor.matmul(psT[:, d * 128:(d + 1) * 128],
                                 raw3[:, t, d * 128:(d + 1) * 128],
                                 diag[:], start=True, stop=True)
            dst = zq3[:, :, t * 128:(t + 1) * 128]
            src = psT[:].rearrange("p (k c) -> p k c", k=_NK)
            nc.vector.tensor_copy(dst, src)

        strip_h = dram.tile([128, _NK * HW], fp8, name=f"strip{h}")
        nc.sync.dma_start(
            strip_h[:].rearrange("p (k w) -> p k w", k=_NK),
            zq3[:, :, h * HW:(h + 1) * HW])
        g = dram.tile([128 * _NCORES, _NK * HW], fp8, name=f"gath{h}")
        nc.gpsimd.collective_compute(
            "AllGather", mybir.AluOpType.bypass,
            replica_groups=[list(range(_NCORES))],
            ins=[strip_h.opt()], outs=[g.opt()])
        gath.append(g)

    # readback: 4 chunks of 2048 cols, one DMA per (chunk, strip)
    for R in range(_N2 // _RBW):
        h, q = R // 2, R % 2
        g3 = gath[h].rearrange("(s2 p) f -> s2 p f", p=128)
        for s in range(4):
            # dst [p][k][w':512]  <-  src rows 128*(4q+s)+p (contiguous k*w)
            dst = zt3[:, :, _RBW * R + 512 * s:_RBW * R + 512 * (s + 1)]
            nc.sync.dma_start(dst, g3[4 * q + s])

    # ---- phase 2.5 (during AG window): self/pos diagonals via PE ----
    # Tiny DR matmuls zq_a.T @ zq_b reproduce the exact fp8-DR arithmetic
    # of the main GEMM; the diagonal is extracted with an identity mask.
    # j<8: self (a=b=j); j>=8: pos (a=j-8, b=j-4 -> rows i vs i+512).
    for j in range(12):
        a = j if j < 8 else j - 8
        b = j if j < 8 else j - 4
        psd = pspool.tile([128, 128], f32, tag="pst", bufs=2,
                          name=f"psd{j}")
        for kk in range(_NK // 2):
            nc.tensor.matmul(
                psd[:], zq3[:, 2 * kk:2 * kk + 2, a * 128:(a + 1) * 128],
                zq3[:, 2 * kk:2 * kk + 2, b * 128:(b + 1) * 128],
                start=(kk == 0), stop=(kk == _NK // 2 - 1), perf_mode=DR)
        junk = pool.tile([128, 128], f32, tag="junk", bufs=2,
                         name=f"junk{j}")
        nc.vector.scalar_tensor_tensor(
            out=junk[:], in0=psd[:], scalar=1.0, in1=ident[:],
            op0=Alu.mult, op1=Alu.mult, accum_out=spv[:, j:j + 1])

    # exp(self) per row block, flavor-matched to the block's exp engine
    nc.scalar.activation(sx[:, 0:6], spv[:, 0:6], Act.Exp,
                         bias=0.0, scale=_SCL)
    sxi = pool.tile([128, 2], i32, tag="sxi")
    nc.vector.tensor_scalar(out=sxi[:], in0=spv[:, 6:8], scalar1=_SCHR_A,
                            scalar2=_SCHR_B, op0=Alu.mult, op1=Alu.add)
    nc.vector.tensor_copy(sx[:, 6:8], sxi[:].bitcast(f32))

    # ---- phase 3: GEMM + exp row-sums ----
    def emit_gemm_group(m, G):
        width = _CGRP[G]
        col0 = sum(_CGRP[:G])
        psfull = pspool.tile([128, max(_CGRP)], f32, tag="ps", bufs=2,
                             name=f"ps{m}_{G}")
        psf = psfull[:, 0:width]
        for kk in range(_NK // 2):
            for c in range(width // 512):
                j = col0 // 512 + c
                nc.tensor.matmul(
                    psf[:, c * 512:(c + 1) * 512],
                    zq3[:, 2 * kk:2 * kk + 2, m * 128:(m + 1) * 128],
                    zt3[:, 2 * kk:2 * kk + 2, j * 512:(j + 1) * 512],
                    start=(kk == 0), stop=(kk == _NK // 2 - 1),
                    perf_mode=DR)
        if m in _DVE_BLOCKS:
            eif = pool.tile([128, max(_CGRP)], i32, tag="ei", bufs=2,
                            name=f"ei{m}_{G}")
            ei = eif[:, 0:width]
            nc.vector.tensor_scalar(out=ei, in0=psf, scalar1=_SCHR_A,
                                    scalar2=_SCHR_B, op0=Alu.mult,
                                    op1=Alu.add)
            nc.vector.reduce_sum(out=se3[:, m, G:G + 1],
                                 in_=ei.bitcast(f32),
                                 axis=mybir.AxisListType.X)
        else:
            nc.scalar.activation(psf, psf, Act.Exp, bias=0.0,
                                 scale=_SCL, accum_out=se3[:, m, G:G + 1])
        if G == _NG - 1:
            nc.vector.reduce_sum(out=rs_all[:, m:m + 1], in_=se3[:, m, :],
                                 axis=mybir.AxisListType.X)

    for G in range(_NG):
        for m in _MORDER:
            emit_gemm_group(m, G)

    # ---- phase 4: lse, loss, partial ----
    sub = pool.tile([128, _NM], f32, tag="sub")
    nc.vector.tensor_sub(sub[:], rs_all[:], sx[:])
    lse = pool.tile([128, _NM], f32, tag="lse")
    nc.scalar.activation(lse[:], sub[:], Act.Ln, bias=0.0, scale=1.0)
    lt = pool.tile([128, 1], f32, tag="lt")
    nc.vector.reduce_sum(out=lt[:], in_=lse[:], axis=mybir.AxisListType.X)
    pv = pool.tile([128, 1], f32, tag="pv")
    nc.vector.reduce_sum(out=pv[:], in_=spv[:, 8:12],
                         axis=mybir.AxisListType.X)
    comb = pool.tile([128, 1], f32, tag="comb")
    nc.vector.scalar_tensor_tensor(
        out=comb[:], in0=pv[:], scalar=-2.0 * _SCL, in1=lt[:],
        op0=Alu.mult, op1=Alu.add)
    pf = pspool.tile([1, 1], f32, tag="pst", bufs=2, name="pf")
    nc.tensor.matmul(pf[:], comb[:], ones[:, 0:1], start=True, stop=True)
    res = pool.tile([1, 1], f32, tag="res")
    nc.vector.tensor_copy(res[:], pf[:])
    nc.sync.dma_start(out_partial[:, :], res[:])

    ctx.close()


def build():
    import concourse.tile as tile
    from concourse import bacc, mybir

    nc = bacc.Bacc("TRN2", target_bir_lowering=False, debug=False,
                   enable_asserts=True, num_devices=_NCORES)
    projs = nc.dram_tensor("projs", [_RPC, _D], mybir.dt.float32,
                           kind="ExternalInput").ap()
    out_partial = nc.dram_tensor("partial", [1, 1], mybir.dt.float32,
                                 kind="ExternalOutput").ap()
    with tile.TileContext(nc) as tc:
        _emit(tc, projs, out_partial)
    nc.compile()
    return nc


_NC_CACHE = None


def _get_nc():
    global _NC_CACHE
    if _NC_CACHE is None:
        _NC_CACHE = build()
    return _NC_CACHE


def make_in_maps(proj_1, proj_2):
    z = np.concatenate([np.asarray(proj_1, dtype=np.float32),
                        np.asarray(proj_2, dtype=np.float32)], axis=0)
    return [{"projs": np.ascontiguousarray(z[c::_NCORES])}
            for c in range(_NCORES)]


def kernel(proj_1, proj_2):
    from concourse import bass_utils

    nc = _get_nc()
    in_maps = make_in_maps(proj_1, proj_2)
    r = bass_utils.run_bass_kernel_spmd(nc, in_maps,
                                        core_ids=list(range(_NCORES)))
    total = sum(float(res["partial"][0, 0]) for res in r.results)
    return np.float32(total / _N2)


# revision 17
# speedup vs baseline: 1.0429x; 1.0429x over previous
"""NT-Xent / contrastive loss on 8 Trainium2 NeuronCores (all-gather design).

Reference computation (B=4096, D=512, temp=0.1):
    z   = l2norm(concat(proj_1, proj_2))          # [8192, 512]
    cos = (z @ z.T) / temp                        # [8192, 8192]
    pos[r]  = cos[r, (r + 4096) % 8192]
    lse[r]  = logsumexp(cos[r, :] with cos[r, r] masked out)
    loss    = mean(lse - pos)

Sharding: CYCLIC rows — core c owns global rows {8*i + c}.  This keeps
(r, r+4096) positive pairs on one core (local rows i and i+512) and makes
the program SPMD-uniform.  Each core receives only its own 1024 rows.

Per core:
  1. Cast-DMA its 8 row-tiles in as bf16; row sumsq (DVE STT);
     1/||row|| via fast-rsqrt + 2 Newton steps (DVE).
  2. Normalize folded into the PE transpose (moving operand diag(rn*S));
     evacuate PSUM to an fp8 strip zq [128, 4k, 1024] (DVE).
  3. AllGather the fp8 strips via DRAM bounce buffers (2 pipelined halves,
     collective runs on TOPSP/SDMA silicon), read back into
     zt [128, 4k, 8192] in 4 chunks.  Column order is a permutation of
     global rows; row sums are permutation-invariant.
  4. GEMM rows x all columns in fp8 DoubleRow (2x PE rate): 8 row-blocks
     x 4 column groups of 2048.  Per group the exp+row-sum runs on:
       - ACT (table Exp with accum_out) for row blocks 0-5,
       - DVE (Schraudolph bit-trick exp + reduce) for blocks 6-7,
     splitting the exp wall across two engines.
  5. Self/pos logits are computed from the LOCAL strip (exact same fp8
     products the GEMM used): prod = zq*zq (bf16), ones-matmuls give
     self[i]=|zq_i|^2 and pos[i]=zq_i.zq_{i+512} in [128, block] layout.
     lse = ln(rowsum - expE(self)) with expE matching the block's engine.
  6. partial = sum(lse) - 2*SCL*sum(pos); host adds partials / 8192.
"""

import sys

import numpy as np

if "/opt/trn_rl_repo" not in sys.path:
    sys.path.insert(0, "/opt/trn_rl_repo")

_B = 4096
_D = 512
_N2 = 2 * _B            # 8192 rows of the similarity matrix
_NCORES = 8
_RPC = _N2 // _NCORES   # 1024 rows per core
_INV_TEMP = 10.0
_S = 64.0               # fp8 pre-scale on normalized rows
_SCL = _INV_TEMP / (_S * _S)   # logit scale applied at exp time

_NT = _RPC // 128       # 8 local row tiles
_NK = _D // 128         # 4 contraction chunks (2 DoubleRow pairs)
_NM = _RPC // 128       # 8 output row blocks per core
_CGRP = (1024, 1024, 1536, 1536, 1536, 1536)   # GEMM column-group widths
_NG = len(_CGRP)
_RBW = 2048             # readback chunk width (4 chunks)
_DVE_BLOCKS = (6, 7)    # row blocks whose exp runs on DVE (Schraudolph)
_MORDER = (0, 6, 1, 2, 7, 3, 4, 5)   # interleave DVE blocks for overlap

_MAGIC1 = 0x5F3759E0    # fast inverse sqrt magic + 1 (M - x == (M+1) + ~x)

# Schraudolph exp: exp(SCL*p) ~= bitcast_f32(int(A*p + Bc))
_SCHR_A = _SCL * (2.0 ** 23) / float(np.log(2.0))
_SCHR_B = 127.0 * 2.0 ** 23 - 486411.0   # ~zero-mean error constant


def _emit(tc, projs, out_partial):
    import concourse.bass as bass  # noqa: F401
    from concourse import mybir

    nc = tc.nc
    f32 = mybir.dt.float32
    bf16 = mybir.dt.bfloat16
    fp8 = mybir.dt.float8e4
    i32 = mybir.dt.int32
    Alu = mybir.AluOpType
    Act = mybir.ActivationFunctionType
    DR = mybir.MatmulPerfMode.DoubleRow

    from contextlib import ExitStack
    ctx = ExitStack()
    pool = ctx.enter_context(tc.tile_pool(name="work", bufs=1))
    pers = ctx.enter_context(tc.tile_pool(name="pers", bufs=1))
    pspool = ctx.enter_context(tc.tile_pool(name="psum", bufs=1, space="PSUM"))
    dram = ctx.enter_context(tc.tile_pool(name="dram", bufs=1, space="DRAM"))

    # ---- input stream first: 8 cast-DMAs on the gpsimd SWDGE queue ----
    raw_all = pers.tile([128, _NT * _D], bf16, tag="raw")
    raw3 = raw_all.rearrange("p (t d) -> p t d", t=_NT)
    projs4 = projs.rearrange("(h t p) d -> h p t d", p=128, t=_NT // 2)
    for h in range(2):
        nc.gpsimd.dma_start(raw3[:, 4 * h:4 * h + 4, :], projs4[h])

    # ---- constants (off the gpsimd queue where possible) ----
    ones = pers.tile([128, 128], f32, tag="ones")
    nc.vector.memset(ones[:], 1.0)
    ident = pers.tile([128, 128], f32, tag="ident")
    nc.gpsimd.affine_select(ident[:], ones[:], pattern=[[1, 128]],
                            compare_op=Alu.is_equal, fill=0.0,
                            base=0, channel_multiplier=-1)
    identb = pers.tile([128, 128], bf16, tag="identb")
    nc.vector.tensor_copy(identb[:], ident[:])

    # ---- persistent buffers ----
    # Half-strip-major layouts: a "half strip" sh is 512 columns of one
    # source core, stored [k:4][w:512] so DMA runs are 2KB contiguous.
    # zq: own strip, h in {0,1};  zt: gathered, sh = 8*h + s.
    zq = pers.tile([128, 2 * _NK * 512], fp8, tag="zq")
    zq4 = zq.rearrange("p (h k w) -> p h k w", h=2, k=_NK)
    zt = pers.tile([128, _NK * _N2], fp8, tag="zt")
    zt2 = zt.rearrange("p (sh f) -> p sh f", sh=16)
    zt4 = zt.rearrange("p (sh k w) -> p sh k w", sh=16, k=_NK)
    se_all = pers.tile([128, _NM * _NG], f32, tag="se")   # group sums
    se3 = se_all.rearrange("p (m g) -> p m g", m=_NM)
    rs_all = pers.tile([128, _NM], f32, tag="rs")         # row sumexp
    spv = pers.tile([128, 16], f32, tag="spv")            # self[0:8] pos[8:12]
    sx = pers.tile([128, _NM], f32, tag="sx")             # exp(self) per block

    # ---- phase 1+2 per half: sumsq -> rsqrt -> transpose/evac -> AG ----
    # Strip half h covers local cols w in [512h, 512h+512) across all k.
    # zt column layout: j = 4096*h + 512*s + w'  (s = source core).
    HW = _RPC // 2   # 512 cols per half
    ss = pool.tile([128, _NT], f32, tag="ss")
    gath = []
    for h in range(2):
        for t in range(4 * h, 4 * h + 4):
            sq = pool.tile([128, _D], bf16, tag="sq", bufs=2, name=f"sq{t}")
            nc.vector.scalar_tensor_tensor(
                out=sq[:], in0=raw3[:, t, :], scalar=1.0, in1=raw3[:, t, :],
                op0=Alu.mult, op1=Alu.mult, accum_out=ss[:, t:t + 1])

        # rnorm = S/sqrt(max(ss, 1e-24)), fast-rsqrt + 2 Newton steps (DVE)
        sh = ss[:, 4 * h:4 * h + 4]
        ssc = pool.tile([128, 4], f32, tag="ssc", bufs=2, name=f"ssc{h}")
        nc.vector.tensor_scalar_max(ssc[:], sh, 1e-24)
        ti = pool.tile([128, 4], i32, tag="ti", bufs=2, name=f"ti{h}")
        nc.vector.tensor_scalar(
            out=ti[:], in0=ssc[:].bitcast(i32), scalar1=1, scalar2=-1,
            op0=Alu.logical_shift_right, op1=Alu.bitwise_xor)
        rn = pool.tile([128, 4], f32, tag="rn", bufs=2, name=f"rn{h}")
        nc.vector.tensor_scalar(
            out=rn[:].bitcast(i32), in0=ti[:], scalar1=_MAGIC1, scalar2=None,
            op0=Alu.add)
        nt = pool.tile([128, 4], f32, tag="nt", bufs=2, name=f"nt{h}")
        for _ in range(2):
            nc.vector.tensor_tensor(out=nt[:], in0=rn[:], in1=rn[:],
                                    op=Alu.mult)
            nc.vector.tensor_tensor(out=nt[:], in0=nt[:], in1=ssc[:],
                                    op=Alu.mult)
            nc.vector.tensor_scalar(out=nt[:], in0=nt[:], scalar1=-0.5,
                                    scalar2=1.5, op0=Alu.mult, op1=Alu.add)
            nc.vector.tensor_tensor(out=rn[:], in0=rn[:], in1=nt[:],
                                    op=Alu.mult)
        rnsc = pool.tile([128, 4], f32, tag="rnsc", bufs=2, name=f"rnsc{h}")
        nc.vector.tensor_scalar_mul(rnsc[:], rn[:], _S)

        for i, t in enumerate(range(4 * h, 4 * h + 4)):
            diag = pool.tile([128, 128], bf16, tag="diag", bufs=2,
                             name=f"dg{t}")
            nc.vector.tensor_scalar_mul(diag[:], identb[:], rnsc[:, i:i + 1])
            psT = pspool.tile([128, _D], f32, tag="pst", bufs=2,
                              name=f"psT{t}")
            for d in range(_NK):
                nc.tensor.matmul(psT[:, d * 128:(d + 1) * 128],
                                 raw3[:, t, d * 128:(d + 1) * 128],
                                 diag[:], start=True, stop=True)
            dst = zq4[:, h, :, i * 128:(i + 1) * 128]
            src = psT[:].rearrange("p (k c) -> p k c", k=_NK)
            nc.vector.tensor_copy(dst, src)

        strip_h = dram.tile([128, _NK * HW], fp8, name=f"strip{h}")
        nc.sync.dma_start(strip_h[:], zq4[:, h].rearrange("p k w -> p (k w)"))
        g = dram.tile([128 * _NCORES, _NK * HW], fp8, name=f"gath{h}",
                      addr_space="Shared")
        nc.gpsimd.collective_compute(
            "AllGather", mybir.AluOpType.bypass,
            replica_groups=[list(range(_NCORES))],
            ins=[strip_h.opt()], outs=[g.opt()])
        gath.append(g)

    # readback: 4 DMAs of 4 half-strips (1MB) each, all runs 2KB contiguous
    for R in range(4):
        h, q = R // 2, R % 2
        g3 = gath[h].rearrange("(s p) f -> s p f", p=128)
        dst = zt2[:, 4 * R:4 * R + 4, :].rearrange("p s f -> p s f")
        src = g3[4 * q:4 * q + 4].rearrange("s p f -> p s f")
        nc.sync.dma_start(dst, src)

    # ---- phase 2.5 (during AG window): self/pos diagonals via PE ----
    # Tiny DR matmuls zq_a.T @ zq_b reproduce the exact fp8-DR arithmetic
    # of the main GEMM; the diagonal is extracted with an identity mask.
    # j<8: self (a=b=j); j>=8: pos (a=j-8, b=j-4 -> rows i vs i+512).
    def zq_block(m, kk):
        return zq4[:, m // 4, 2 * kk:2 * kk + 2,
                   (m % 4) * 128:(m % 4 + 1) * 128]

    for j in range(12):
        a = j if j < 8 else j - 8
        b = j if j < 8 else j - 4
        psd = pspool.tile([128, 128], f32, tag="pst", bufs=2,
                          name=f"psd{j}")
        for kk in range(_NK // 2):
            nc.tensor.matmul(
                psd[:], zq_block(a, kk), zq_block(b, kk),
                start=(kk == 0), stop=(kk == _NK // 2 - 1), perf_mode=DR)
        junk = pool.tile([128, 128], f32, tag="junk", bufs=2,
                         name=f"junk{j}")
        nc.vector.scalar_tensor_tensor(
            out=junk[:], in0=psd[:], scalar=1.0, in1=ident[:],
            op0=Alu.mult, op1=Alu.mult, accum_out=spv[:, j:j + 1])

    # exp(self) per row block, flavor-matched to the block's exp engine
    nc.scalar.activation(sx[:, 0:6], spv[:, 0:6], Act.Exp,
                         bias=0.0, scale=_SCL)
    sxi = pool.tile([128, 2], i32, tag="sxi")
    nc.vector.tensor_scalar(out=sxi[:], in0=spv[:, 6:8], scalar1=_SCHR_A,
                            scalar2=_SCHR_B, op0=Alu.mult, op1=Alu.add)
    nc.vector.tensor_copy(sx[:, 6:8], sxi[:].bitcast(f32))

    # ---- phase 3: GEMM + exp row-sums ----
    def emit_gemm_group(m, G):
        width = _CGRP[G]
        col0 = sum(_CGRP[:G])
        psfull = pspool.tile([128, max(_CGRP)], f32, tag="ps", bufs=2,
                             name=f"ps{m}_{G}")
        psf = psfull[:, 0:width]
        for kk in range(_NK // 2):
            for c in range(width // 512):
                j = col0 // 512 + c
                nc.tensor.matmul(
                    psf[:, c * 512:(c + 1) * 512],
                    zq_block(m, kk),
                    zt4[:, j, 2 * kk:2 * kk + 2, :],
                    start=(kk == 0), stop=(kk == _NK // 2 - 1),
                    perf_mode=DR)
        if m in _DVE_BLOCKS:
            eif = pool.tile([128, max(_CGRP)], i32, tag="ei", bufs=2,
                            name=f"ei{m}_{G}")
            ei = eif[:, 0:width]
            nc.vector.tensor_scalar(out=ei, in0=psf, scalar1=_SCHR_A,
                                    scalar2=_SCHR_B, op0=Alu.mult,
                                    op1=Alu.add)
            nc.vector.reduce_sum(out=se3[:, m, G:G + 1],
                                 in_=ei.bitcast(f32),
                                 axis=mybir.AxisListType.X)
        else:
            nc.scalar.activation(psf, psf, Act.Exp, bias=0.0,
                                 scale=_SCL, accum_out=se3[:, m, G:G + 1])
        if G == _NG - 1:
            nc.vector.reduce_sum(out=rs_all[:, m:m + 1], in_=se3[:, m, :],
                                 axis=mybir.AxisListType.X)

    for G in range(_NG):
        for m in _MORDER:
            emit_gemm_group(m, G)

    # ---- phase 4: lse, loss, partial ----
    sub = pool.tile([128, _NM], f32, tag="sub")
    nc.vector.tensor_sub(sub[:], rs_all[:], sx[:])
    lse = pool.tile([128, _NM], f32, tag="lse")
    nc.scalar.activation(lse[:], sub[:], Act.Ln, bias=0.0, scale=1.0)
    lt = pool.tile([128, 1], f32, tag="lt")
    nc.vector.reduce_sum(out=lt[:], in_=lse[:], axis=mybir.AxisListType.X)
    pv = pool.tile([128, 1], f32, tag="pv")
    nc.vector.reduce_sum(out=pv[:], in_=spv[:, 8:12],
                         axis=mybir.AxisListType.X)
    comb = pool.tile([128, 1], f32, tag="comb")
    nc.vector.scalar_tensor_tensor(
        out=comb[:], in0=pv[:], scalar=-2.0 * _SCL, in1=lt[:],
        op0=Alu.mult, op1=Alu.add)
    pf = pspool.tile([1, 1], f32, tag="pst", bufs=2, name="pf")
    nc.tensor.matmul(pf[:], comb[:], ones[:, 0:1], start=True, stop=True)
    res = pool.tile([1, 1], f32, tag="res")
    nc.vector.tensor_copy(res[:], pf[:])
    nc.sync.dma_start(out_partial[:, :], res[:])

    ctx.close()


def build():
    import concourse.tile as tile
    from concourse import bacc, mybir

    nc = bacc.Bacc("TRN2", target_bir_lowering=False, debug=False,
                   enable_asserts=True, num_devices=_NCORES)
    projs = nc.dram_tensor("projs", [_RPC, _D], mybir.dt.float32,
                           kind="ExternalInput").ap()
    out_partial = nc.dram_tensor("partial", [1, 1], mybir.dt.float32,
                                 kind="ExternalOutput").ap()
    with tile.TileContext(nc) as tc:
        _emit(tc, projs, out_partial)
    nc.compile()
    return nc


_NC_CACHE = None


def _get_nc():
    global _NC_CACHE
    if _NC_CACHE is None:
        _NC_CACHE = build()
    return _NC_CACHE


def make_in_maps(proj_1, proj_2):
    z = np.concatenate([np.asarray(proj_1, dtype=np.float32),
                        np.asarray(proj_2, dtype=np.float32)], axis=0)
    return [{"projs": np.ascontiguousarray(z[c::_NCORES])}
            for c in range(_NCORES)]


def kernel(proj_1, proj_2):
    from concourse import bass_utils

    nc = _get_nc()
    in_maps = make_in_maps(proj_1, proj_2)
    r = bass_utils.run_bass_kernel_spmd(nc, in_maps,
                                        core_ids=list(range(_NCORES)))
    total = sum(float(res["partial"][0, 0]) for res in r.results)
    return np.float32(total / _N2)


# revision 19
# speedup vs baseline: 1.1551x; 1.1075x over previous
"""NT-Xent / contrastive loss on 8 Trainium2 NeuronCores.

Reference computation (B=4096, D=512, temp=0.1):
    z   = l2norm(concat(proj_1, proj_2))          # [8192, 512]
    cos = (z @ z.T) / temp                        # [8192, 8192]
    pos[r]  = cos[r, (r + 4096) % 8192]
    lse[r]  = logsumexp(cos[r, :] with cos[r, r] masked out)
    loss    = mean(lse - pos)

Sharding: rows of the similarity matrix, 1024 per core.  Each core
receives the full stacked [8192, 512] input *rotated* by core*1024 rows,
which makes the program uniform across cores (SPMD): local rows 0..1023
are the core's rows, the self-diagonal sits at local column == row, and
the positive sits at local column == row + 4096.

Per core:
  1. SWDGE cast-DMA streams the input in as bf16 (f32 read from HBM,
     bf16 write to SBUF), two row-tiles per descriptor batch.
  2. Row sumsq via DVE scalar_tensor_tensor (bf16); 1/||row|| via
     fast-rsqrt (int magic) + 2 Newton steps on DVE.
  3. Normalization folded into the PE transpose (moving operand is
     diag(rn*S)); PSUM evacuated to fp8 alternately by DVE tensor_copy
     and ACT Copy-activation to split the evacuation wall.
  4. GEMM in fp8 DoubleRow (2x PE): 8 row blocks x column groups
     (1024,1024,1536x4).  Exp + row-sum per group runs on ACT (table
     Exp, accum_out) for 7 blocks and on DVE for block 7 via a
     Schraudolph bit-trick exp (tensor_scalar to int + reduce), which
     splits the exp wall across two engines.
  5. Self/pos logits come from dedicated fp8-DR matmuls on the local
     column blocks (diag extracted with an identity mask) so they are
     bit-consistent with the main GEMM; exp(self) per block uses the
     same exp flavor as that block's row sums.
     lse = ln(rowsum - expE(self)); partial = sum(lse - SCL*pos).
Host adds the 8 partials and divides by 8192.
"""

import sys

import numpy as np

if "/opt/trn_rl_repo" not in sys.path:
    sys.path.insert(0, "/opt/trn_rl_repo")

_B = 4096
_D = 512
_N2 = 2 * _B            # 8192 rows of the similarity matrix
_NCORES = 8
_RPC = _N2 // _NCORES   # 1024 rows per core
_INV_TEMP = 10.0
_S = 64.0               # fp8 pre-scale on normalized rows
_SCL = _INV_TEMP / (_S * _S)   # logit scale applied at exp time

_NT = _N2 // 128        # 64 input row-tiles
_NK = _D // 128         # 4 contraction chunks (2 DoubleRow pairs)
_NM = _RPC // 128       # 8 output row blocks per core
_CGRP = (1024, 1024, 1536, 1536, 1536, 1536)   # GEMM column-group widths
_NG = len(_CGRP)
_DVE_BLOCKS = (7,)      # row blocks whose exp runs on DVE (Schraudolph)
_MORDER = (0, 1, 7, 2, 3, 4, 5, 6)   # interleave the DVE block for overlap
_BATCH = 8              # tiles per load/rsqrt batch

_MAGIC1 = 0x5F3759E0    # fast inverse sqrt magic + 1 (M - x == (M+1) + ~x)

# Schraudolph exp: exp(SCL*p) ~= bitcast_f32(int(A*p + Bc))
_SCHR_A = _SCL * (2.0 ** 23) / float(np.log(2.0))
_SCHR_B = 127.0 * 2.0 ** 23 - 486411.0   # ~zero-mean error constant


def _emit(tc, projs, out_partial):
    import concourse.bass as bass  # noqa: F401
    from concourse import mybir

    nc = tc.nc
    f32 = mybir.dt.float32
    bf16 = mybir.dt.bfloat16
    fp8 = mybir.dt.float8e4
    i32 = mybir.dt.int32
    Alu = mybir.AluOpType
    Act = mybir.ActivationFunctionType
    DR = mybir.MatmulPerfMode.DoubleRow

    from contextlib import ExitStack
    ctx = ExitStack()
    pool = ctx.enter_context(tc.tile_pool(name="work", bufs=1))
    pers = ctx.enter_context(tc.tile_pool(name="pers", bufs=1))
    pspool = ctx.enter_context(tc.tile_pool(name="psum", bufs=1, space="PSUM"))

    # ---- input stream first: cast-DMAs on the gpsimd SWDGE queue ----
    raw_all = pers.tile([128, _NT * _D], bf16, tag="raw")
    raw3 = raw_all.rearrange("p (t d) -> p t d", t=_NT)
    projs3 = projs.rearrange("(t p) d -> p t d", p=128)
    for t2 in range(_NT // 2):
        nc.gpsimd.dma_start(raw3[:, 2 * t2:2 * t2 + 2, :],
                            projs3[:, 2 * t2:2 * t2 + 2, :])

    # ---- constants ----
    ones = pers.tile([128, 128], f32, tag="ones")
    nc.vector.memset(ones[:], 1.0)
    ident = pers.tile([128, 128], f32, tag="ident")
    nc.gpsimd.affine_select(ident[:], ones[:], pattern=[[1, 128]],
                            compare_op=Alu.is_equal, fill=0.0,
                            base=0, channel_multiplier=-1)
    identb = pers.tile([128, 128], bf16, tag="identb")
    nc.vector.tensor_copy(identb[:], ident[:])

    # ---- persistent buffers ----
    zt = pers.tile([128, _NK * _N2], fp8, tag="zt")       # z.T * rn * S, fp8
    zt3 = zt.rearrange("p (k c) -> p k c", k=_NK)
    se_all = pers.tile([128, _NM * _NG], f32, tag="se")   # group sums
    se3 = se_all.rearrange("p (m g) -> p m g", m=_NM)
    rs_all = pers.tile([128, _NM], f32, tag="rs")         # row sumexp
    spv = pers.tile([128, 16], f32, tag="spv")            # self[0:8] pos[8:16]
    sx = pers.tile([128, _NM], f32, tag="sx")             # exp(self) per block

    # ---- phase 1 per batch: sumsq -> rsqrt -> diag/transpose/evac ----
    def emit_load_group(g):
        t0 = g * _BATCH
        ss = pool.tile([128, _BATCH], f32, tag="ss", bufs=2, name=f"ss{g}")
        for i in range(_BATCH):
            t = t0 + i
            sq = pool.tile([128, _D], bf16, tag="sq", bufs=2, name=f"sq{t}")
            nc.vector.scalar_tensor_tensor(
                out=sq[:], in0=raw3[:, t, :], scalar=1.0, in1=raw3[:, t, :],
                op0=Alu.mult, op1=Alu.mult, accum_out=ss[:, i:i + 1])

        # rnorm = S/sqrt(max(ss, 1e-24)), fast-rsqrt + 2 Newton steps (DVE)
        ssc = pool.tile([128, _BATCH], f32, tag="ssc", bufs=2, name=f"ssc{g}")
        nc.vector.tensor_scalar_max(ssc[:], ss[:], 1e-24)
        ti = pool.tile([128, _BATCH], i32, tag="ti", bufs=2, name=f"ti{g}")
        nc.vector.tensor_scalar(
            out=ti[:], in0=ssc[:].bitcast(i32), scalar1=1, scalar2=-1,
            op0=Alu.logical_shift_right, op1=Alu.bitwise_xor)
        rn = pool.tile([128, _BATCH], f32, tag="rn", bufs=2, name=f"rn{g}")
        nc.vector.tensor_scalar(
            out=rn[:].bitcast(i32), in0=ti[:], scalar1=_MAGIC1, scalar2=None,
            op0=Alu.add)
        nt = pool.tile([128, _BATCH], f32, tag="nt", bufs=2, name=f"nt{g}")
        for _ in range(2):
            nc.vector.tensor_tensor(out=nt[:], in0=rn[:], in1=rn[:],
                                    op=Alu.mult)
            nc.vector.tensor_tensor(out=nt[:], in0=nt[:], in1=ssc[:],
                                    op=Alu.mult)
            nc.vector.tensor_scalar(out=nt[:], in0=nt[:], scalar1=-0.5,
                                    scalar2=1.5, op0=Alu.mult, op1=Alu.add)
            nc.vector.tensor_tensor(out=rn[:], in0=rn[:], in1=nt[:],
                                    op=Alu.mult)
        rnsc = pool.tile([128, _BATCH], f32, tag="rnsc", bufs=2,
                         name=f"rnsc{g}")
        nc.vector.tensor_scalar_mul(rnsc[:], rn[:], _S)

        for i in range(_BATCH):
            t = t0 + i
            diag = pool.tile([128, 128], bf16, tag="diag", bufs=4,
                             name=f"dg{t}")
            nc.vector.tensor_scalar_mul(diag[:], identb[:], rnsc[:, i:i + 1])
            psT = pspool.tile([128, _D], f32, tag="pst", bufs=2,
                              name=f"psT{t}")
            for d in range(_NK):
                nc.tensor.matmul(psT[:, d * 128:(d + 1) * 128],
                                 raw3[:, t, d * 128:(d + 1) * 128],
                                 diag[:], start=True, stop=True)
            dst = zt3[:, :, t * 128:(t + 1) * 128]
            src = psT[:].rearrange("p (k c) -> p k c", k=_NK)
            if t % 2 == 0:
                nc.vector.tensor_copy(dst, src)
            else:
                nc.scalar.activation(dst, src, Act.Copy, bias=0.0, scale=1.0)

    # ---- self/pos diagonals via PE (bit-consistent with the GEMM) ----
    # j<8: self (a=b=j); j>=8: pos (a=j-8, b=32+j-8 -> col r+4096).
    def emit_selfpos(j):
        a = j if j < 8 else j - 8
        b = j if j < 8 else 32 + (j - 8)
        psd = pspool.tile([128, 128], f32, tag="pst", bufs=2, name=f"psd{j}")
        for kk in range(_NK // 2):
            nc.tensor.matmul(
                psd[:], zt3[:, 2 * kk:2 * kk + 2, a * 128:(a + 1) * 128],
                zt3[:, 2 * kk:2 * kk + 2, b * 128:(b + 1) * 128],
                start=(kk == 0), stop=(kk == _NK // 2 - 1), perf_mode=DR)
        junk = pool.tile([128, 128], f32, tag="junk", bufs=2, name=f"jk{j}")
        nc.vector.scalar_tensor_tensor(
            out=junk[:], in0=psd[:], scalar=1.0, in1=ident[:],
            op0=Alu.mult, op1=Alu.mult, accum_out=spv[:, j:j + 1])

    # ---- GEMM group + exp row-sum ----
    def emit_gemm_group(m, G):
        width = _CGRP[G]
        col0 = sum(_CGRP[:G])
        psfull = pspool.tile([128, max(_CGRP)], f32, tag="ps", bufs=2,
                             name=f"ps{m}_{G}")
        psf = psfull[:, 0:width]
        for kk in range(_NK // 2):
            for c in range(width // 512):
                j = col0 // 512 + c
                nc.tensor.matmul(
                    psf[:, c * 512:(c + 1) * 512],
                    zt3[:, 2 * kk:2 * kk + 2, m * 128:(m + 1) * 128],
                    zt3[:, 2 * kk:2 * kk + 2, j * 512:(j + 1) * 512],
                    start=(kk == 0), stop=(kk == _NK // 2 - 1),
                    perf_mode=DR)
        if m in _DVE_BLOCKS:
            eif = pool.tile([128, max(_CGRP)], i32, tag="ei", bufs=2,
                            name=f"ei{m}_{G}")
            ei = eif[:, 0:width]
            nc.vector.tensor_scalar(out=ei, in0=psf, scalar1=_SCHR_A,
                                    scalar2=_SCHR_B, op0=Alu.mult,
                                    op1=Alu.add)
            nc.vector.reduce_sum(out=se3[:, m, G:G + 1],
                                 in_=ei.bitcast(f32),
                                 axis=mybir.AxisListType.X)
        else:
            nc.scalar.activation(psf, psf, Act.Exp, bias=0.0,
                                 scale=_SCL, accum_out=se3[:, m, G:G + 1])
        if G == _NG - 1:
            nc.vector.reduce_sum(out=rs_all[:, m:m + 1], in_=se3[:, m, :],
                                 axis=mybir.AxisListType.X)

    # ---- interleaved emission: stream tiles, fire GEMM groups when fed ----
    next_g = 0
    sp_done = False

    for G in range(_NG):
        need = (sum(_CGRP[:G + 1]) + 127) // 128
        need = max(need, 8)  # lhs panel: tiles 0..7
        while next_g * _BATCH < need:
            emit_load_group(next_g)
            next_g += 1
        if not sp_done and next_g * _BATCH >= 40:
            for j in range(16):
                emit_selfpos(j)
            # exp(self), flavor-matched per block
            nc.scalar.activation(sx[:, 0:7], spv[:, 0:7], Act.Exp,
                                 bias=0.0, scale=_SCL)
            sxi = pool.tile([128, 1], i32, tag="sxi")
            nc.vector.tensor_scalar(out=sxi[:], in0=spv[:, 7:8],
                                    scalar1=_SCHR_A, scalar2=_SCHR_B,
                                    op0=Alu.mult, op1=Alu.add)
            nc.vector.tensor_copy(sx[:, 7:8], sxi[:].bitcast(f32))
            sp_done = True
        for m in _MORDER:
            emit_gemm_group(m, G)
    while next_g * _BATCH < _NT:
        emit_load_group(next_g)
        next_g += 1

    # ---- final: lse, loss, partial ----
    sub = pool.tile([128, _NM], f32, tag="sub")
    nc.vector.tensor_sub(sub[:], rs_all[:], sx[:])
    lse = pool.tile([128, _NM], f32, tag="lse")
    nc.scalar.activation(lse[:], sub[:], Act.Ln, bias=0.0, scale=1.0)
    lt = pool.tile([128, 1], f32, tag="lt")
    nc.vector.reduce_sum(out=lt[:], in_=lse[:], axis=mybir.AxisListType.X)
    pv = pool.tile([128, 1], f32, tag="pv")
    nc.vector.reduce_sum(out=pv[:], in_=spv[:, 8:16],
                         axis=mybir.AxisListType.X)
    comb = pool.tile([128, 1], f32, tag="comb")
    nc.vector.scalar_tensor_tensor(
        out=comb[:], in0=pv[:], scalar=-_SCL, in1=lt[:],
        op0=Alu.mult, op1=Alu.add)
    pf = pspool.tile([1, 1], f32, tag="pst", bufs=2, name="pf")
    nc.tensor.matmul(pf[:], comb[:], ones[:, 0:1], start=True, stop=True)
    res = pool.tile([1, 1], f32, tag="res")
    nc.vector.tensor_copy(res[:], pf[:])
    nc.sync.dma_start(out_partial[:, :], res[:])

    ctx.close()


def build():
    import concourse.tile as tile
    from concourse import bacc, mybir

    nc = bacc.Bacc("TRN2", target_bir_lowering=False, debug=False,
                   enable_asserts=True, num_devices=_NCORES)
    projs = nc.dram_tensor("projs", [_N2, _D], mybir.dt.float32,
                           kind="ExternalInput").ap()
    out_partial = nc.dram_tensor("partial", [1, 1], mybir.dt.float32,
                                 kind="ExternalOutput").ap()
    with tile.TileContext(nc) as tc:
        _emit(tc, projs, out_partial)
    nc.compile()
    return nc


_NC_CACHE = None


def _get_nc():
    global _NC_CACHE
    if _NC_CACHE is None:
        _NC_CACHE = build()
    return _NC_CACHE


def make_in_maps(proj_1, proj_2):
    z = np.concatenate([np.asarray(proj_1, dtype=np.float32),
                        np.asarray(proj_2, dtype=np.float32)], axis=0)
    return [{"projs": np.ascontiguousarray(np.roll(z, -_RPC * c, axis=0))}
            for c in range(_NCORES)]


def kernel(proj_1, proj_2):
    from concourse import bass_utils

    nc = _get_nc()
    in_maps = make_in_maps(proj_1, proj_2)
    r = bass_utils.run_bass_kernel_spmd(nc, in_maps,
                                        core_ids=list(range(_NCORES)))
    total = sum(float(res["partial"][0, 0]) for res in r.results)
    return np.float32(total / _N2)


# revision 28
# speedup vs baseline: 1.2533x; 1.0850x over previous
"""NT-Xent / contrastive loss on 8 Trainium2 NeuronCores.

Reference computation (B=4096, D=512, temp=0.1):
    z   = l2norm(concat(proj_1, proj_2))          # [8192, 512]
    cos = (z @ z.T) / temp                        # [8192, 8192]
    pos[r]  = cos[r, (r + 4096) % 8192]
    lse[r]  = logsumexp(cos[r, :] with cos[r, r] masked out)
    loss    = mean(lse - pos)

Sharding: rows of the similarity matrix, 1024 per core.  Each core
receives the full stacked [8192, 512] input *rotated* by core*1024 rows,
which makes the program uniform across cores (SPMD): local rows 0..1023
are the core's rows, the self-diagonal sits at local column == row, and
the positive sits at local column == row + 4096.

Per core:
  1. SWDGE cast-DMA streams the input in as bf16 (f32 read from HBM,
     bf16 write to SBUF), two row-tiles per descriptor batch.
  2. Row sumsq via DVE scalar_tensor_tensor (bf16); 1/||row|| via
     fast-rsqrt (int magic) + 2 Newton steps on DVE.
  3. Normalization folded into the PE transpose (moving operand is
     diag(rn*S)); PSUM evacuated to fp8 alternately by DVE tensor_copy
     and ACT Copy-activation to split the evacuation wall.
  4. GEMM in fp8 DoubleRow (2x PE): 8 row blocks x column groups
     (1024,1024,1536x4).  Exp + row-sum per group runs on ACT (table
     Exp, accum_out) for 7 blocks and on DVE for block 7 via a
     Schraudolph bit-trick exp (tensor_scalar to int + reduce), which
     splits the exp wall across two engines.
  5. Self/pos logits come from dedicated fp8-DR matmuls on the local
     column blocks (diag extracted with an identity mask) so they are
     bit-consistent with the main GEMM; exp(self) per block uses the
     same exp flavor as that block's row sums.
     lse = ln(rowsum - expE(self)); partial = sum(lse - SCL*pos).
Host adds the 8 partials and divides by 8192.
"""

import sys

import numpy as np

if "/opt/trn_rl_repo" not in sys.path:
    sys.path.insert(0, "/opt/trn_rl_repo")

_B = 4096
_D = 512
_N2 = 2 * _B            # 8192 rows of the similarity matrix
_NCORES = 8
_RPC = _N2 // _NCORES   # 1024 rows per core
_INV_TEMP = 10.0
_S = 64.0               # fp8 pre-scale on normalized rows
_SCL = _INV_TEMP / (_S * _S)   # logit scale applied at exp time

_NT = _N2 // 128        # 64 input row-tiles
_NK = _D // 128         # 4 contraction chunks (2 DoubleRow pairs)
_NM = _RPC // 128       # 8 output row blocks per core
_CGRP = (1024, 1024, 1536, 1536, 1536, 1536)   # GEMM column-group widths
_NG = len(_CGRP)
_DVE_BLOCKS = (7,)      # row blocks whose exp runs on DVE (Schraudolph)
_MORDER = (0, 1, 7, 2, 3, 4, 5, 6)   # interleave the DVE block for overlap
_BATCH = 8              # tiles per load/rsqrt batch

_MAGIC1 = 0x5F3759E0    # fast inverse sqrt magic + 1 (M - x == (M+1) + ~x)

# Schraudolph exp: exp(SCL*p) ~= bitcast_f32(int(A*p + Bc))
_SCHR_A = _SCL * (2.0 ** 23) / float(np.log(2.0))
_SCHR_B = 127.0 * 2.0 ** 23 - 486411.0   # ~zero-mean error constant


def _emit(tc, projs, consts, out_partial):
    import concourse.bass as bass  # noqa: F401
    from concourse import mybir

    nc = tc.nc
    f32 = mybir.dt.float32
    bf16 = mybir.dt.bfloat16
    fp8 = mybir.dt.float8e4
    i32 = mybir.dt.int32
    Alu = mybir.AluOpType
    Act = mybir.ActivationFunctionType
    DR = mybir.MatmulPerfMode.DoubleRow

    from contextlib import ExitStack
    ctx = ExitStack()
    pool = ctx.enter_context(tc.tile_pool(name="work", bufs=1))
    pers = ctx.enter_context(tc.tile_pool(name="pers", bufs=1))
    pspool = ctx.enter_context(tc.tile_pool(name="psum", bufs=1, space="PSUM"))

    # ---- input stream first: cast-DMAs on the gpsimd SWDGE queue ----
    raw_all = pers.tile([128, _NT * _D], bf16, tag="raw")
    raw3 = raw_all.rearrange("p (t d) -> p t d", t=_NT)
    projs3 = projs.rearrange("(t p) d -> p t d", p=128)
    for t2 in range(_NT // 2):
        nc.gpsimd.dma_start(raw3[:, 2 * t2:2 * t2 + 2, :],
                            projs3[:, 2 * t2:2 * t2 + 2, :])

    # ---- constants (host-provided: eye | ones column) ----
    cons = pers.tile([128, 129], f32, tag="cons")
    nc.sync.dma_start(cons[:], consts[:])
    ident = cons[:, 0:128]
    ones = cons[:, 128:129]
    identb = pers.tile([128, 128], bf16, tag="identb")
    nc.vector.tensor_copy(identb[:], ident)

    # ---- persistent buffers ----
    zt = pers.tile([128, _NK * _N2], fp8, tag="zt")       # z.T * rn * S, fp8
    zt3 = zt.rearrange("p (k c) -> p k c", k=_NK)
    se_all = pers.tile([128, _NM * _NG], f32, tag="se")   # group sums
    se3 = se_all.rearrange("p (m g) -> p m g", m=_NM)
    rs_all = pers.tile([128, _NM], f32, tag="rs")         # row sumexp
    spv = pers.tile([128, 16], f32, tag="spv")            # self[0:8] pos[8:16]
    sx = pers.tile([128, _NM], f32, tag="sx")             # exp(self) per block

    # ---- phase 1 per batch: sumsq -> rsqrt -> diag/transpose/evac ----
    def emit_load_group(g):
        t0 = g * _BATCH
        ss = pool.tile([128, _BATCH], f32, tag="ss", bufs=2, name=f"ss{g}")
        for i in range(_BATCH):
            t = t0 + i
            sq = pool.tile([128, _D], bf16, tag="sq", bufs=4, name=f"sq{t}")
            nc.vector.scalar_tensor_tensor(
                out=sq[:], in0=raw3[:, t, :], scalar=1.0, in1=raw3[:, t, :],
                op0=Alu.mult, op1=Alu.mult, accum_out=ss[:, i:i + 1])

        # rnorm = S/sqrt(max(ss, 1e-24)), fast-rsqrt + 2 Newton steps (DVE)
        ssc = pool.tile([128, _BATCH], f32, tag="ssc", bufs=2, name=f"ssc{g}")
        nc.vector.tensor_scalar_max(ssc[:], ss[:], 1e-24)
        ti = pool.tile([128, _BATCH], i32, tag="ti", bufs=2, name=f"ti{g}")
        nc.vector.tensor_scalar(
            out=ti[:], in0=ssc[:].bitcast(i32), scalar1=1, scalar2=-1,
            op0=Alu.logical_shift_right, op1=Alu.bitwise_xor)
        rn = pool.tile([128, _BATCH], f32, tag="rn", bufs=2, name=f"rn{g}")
        nc.vector.tensor_scalar(
            out=rn[:].bitcast(i32), in0=ti[:], scalar1=_MAGIC1, scalar2=None,
            op0=Alu.add)
        nt = pool.tile([128, _BATCH], f32, tag="nt", bufs=2, name=f"nt{g}")
        for _ in range(2):
            nc.vector.tensor_tensor(out=nt[:], in0=rn[:], in1=rn[:],
                                    op=Alu.mult)
            nc.vector.tensor_tensor(out=nt[:], in0=nt[:], in1=ssc[:],
                                    op=Alu.mult)
            nc.vector.tensor_scalar(out=nt[:], in0=nt[:], scalar1=-0.5,
                                    scalar2=1.5, op0=Alu.mult, op1=Alu.add)
            nc.vector.tensor_tensor(out=rn[:], in0=rn[:], in1=nt[:],
                                    op=Alu.mult)
        rnsc = pool.tile([128, _BATCH], f32, tag="rnsc", bufs=2,
                         name=f"rnsc{g}")
        nc.vector.tensor_scalar_mul(rnsc[:], rn[:], _S)

        for i in range(_BATCH):
            t = t0 + i
            diag = pool.tile([128, 128], bf16, tag="diag", bufs=4,
                             name=f"dg{t}")
            nc.vector.tensor_scalar_mul(diag[:], identb[:], rnsc[:, i:i + 1])
            psT = pspool.tile([128, _D], f32, tag="pst", bufs=2,
                              name=f"psT{t}")
            for d in range(_NK):
                nc.tensor.matmul(psT[:, d * 128:(d + 1) * 128],
                                 raw3[:, t, d * 128:(d + 1) * 128],
                                 diag[:], start=True, stop=True)
            dst = zt3[:, :, t * 128:(t + 1) * 128]
            src = psT[:].rearrange("p (k c) -> p k c", k=_NK)
            if t % 2 == 0:
                nc.vector.tensor_copy(dst, src)
            else:
                nc.scalar.activation(dst, src, Act.Copy, bias=0.0, scale=1.0)

    # ---- self/pos diagonals via PE (bit-consistent with the GEMM) ----
    # j<8: self (a=b=j); j>=8: pos (a=j-8, b=32+j-8 -> col r+4096).
    def emit_selfpos(j):
        a = j if j < 8 else j - 8
        b = j if j < 8 else 32 + (j - 8)
        psd = pspool.tile([128, 128], f32, tag="pst", bufs=2, name=f"psd{j}")
        for kk in range(_NK // 2):
            nc.tensor.matmul(
                psd[:], zt3[:, 2 * kk:2 * kk + 2, a * 128:(a + 1) * 128],
                zt3[:, 2 * kk:2 * kk + 2, b * 128:(b + 1) * 128],
                start=(kk == 0), stop=(kk == _NK // 2 - 1), perf_mode=DR)
        junk = pool.tile([128, 128], f32, tag="junk", bufs=2, name=f"jk{j}")
        nc.vector.scalar_tensor_tensor(
            out=junk[:], in0=psd[:], scalar=1.0, in1=ident[:],
            op0=Alu.mult, op1=Alu.mult, accum_out=spv[:, j:j + 1])

    # ---- GEMM group + exp row-sum ----
    def emit_gemm_group(m, G):
        width = _CGRP[G]
        col0 = sum(_CGRP[:G])
        psfull = pspool.tile([128, max(_CGRP)], f32, tag="ps", bufs=2,
                             name=f"ps{m}_{G}")
        psf = psfull[:, 0:width]
        for kk in range(_NK // 2):
            for c in range(width // 512):
                j = col0 // 512 + c
                nc.tensor.matmul(
                    psf[:, c * 512:(c + 1) * 512],
                    zt3[:, 2 * kk:2 * kk + 2, m * 128:(m + 1) * 128],
                    zt3[:, 2 * kk:2 * kk + 2, j * 512:(j + 1) * 512],
                    start=(kk == 0), stop=(kk == _NK // 2 - 1),
                    perf_mode=DR)
        if m in _DVE_BLOCKS:
            eif = pool.tile([128, max(_CGRP)], i32, tag="ei", bufs=2,
                            name=f"ei{m}_{G}")
            ei = eif[:, 0:width]
            nc.vector.tensor_scalar(out=ei, in0=psf, scalar1=_SCHR_A,
                                    scalar2=_SCHR_B, op0=Alu.mult,
                                    op1=Alu.add)
            nc.vector.reduce_sum(out=se3[:, m, G:G + 1],
                                 in_=ei.bitcast(f32),
                                 axis=mybir.AxisListType.X)
        else:
            nc.scalar.activation(psf, psf, Act.Exp, bias=0.0,
                                 scale=_SCL, accum_out=se3[:, m, G:G + 1])
        if G == _NG - 1:
            nc.vector.reduce_sum(out=rs_all[:, m:m + 1], in_=se3[:, m, :],
                                 axis=mybir.AxisListType.X)

    # ---- interleaved emission: stream tiles, fire GEMM groups when fed ----
    next_g = 0
    sp_done = False

    for G in range(_NG):
        need = (sum(_CGRP[:G + 1]) + 127) // 128
        need = max(need, 8)  # lhs panel: tiles 0..7
        while next_g * _BATCH < need:
            emit_load_group(next_g)
            next_g += 1
        if not sp_done and next_g * _BATCH >= 40:
            for j in range(16):
                emit_selfpos(j)
            # exp(self), flavor-matched per block
            nc.scalar.activation(sx[:, 0:7], spv[:, 0:7], Act.Exp,
                                 bias=0.0, scale=_SCL)
            sxi = pool.tile([128, 1], i32, tag="sxi")
            nc.vector.tensor_scalar(out=sxi[:], in0=spv[:, 7:8],
                                    scalar1=_SCHR_A, scalar2=_SCHR_B,
                                    op0=Alu.mult, op1=Alu.add)
            nc.vector.tensor_copy(sx[:, 7:8], sxi[:].bitcast(f32))
            sp_done = True
        for m in _MORDER:
            emit_gemm_group(m, G)
    while next_g * _BATCH < _NT:
        emit_load_group(next_g)
        next_g += 1

    # ---- final: lse, loss, partial ----
    sub = pool.tile([128, _NM], f32, tag="sub")
    nc.vector.tensor_sub(sub[:], rs_all[:], sx[:])
    lse = pool.tile([128, _NM], f32, tag="lse")
    nc.scalar.activation(lse[:], sub[:], Act.Ln, bias=0.0, scale=1.0)
    lt = pool.tile([128, 1], f32, tag="lt")
    nc.vector.reduce_sum(out=lt[:], in_=lse[:], axis=mybir.AxisListType.X)
    pv = pool.tile([128, 1], f32, tag="pv")
    nc.vector.reduce_sum(out=pv[:], in_=spv[:, 8:16],
                         axis=mybir.AxisListType.X)
    comb = pool.tile([128, 1], f32, tag="comb")
    nc.vector.scalar_tensor_tensor(
        out=comb[:], in0=pv[:], scalar=-_SCL, in1=lt[:],
        op0=Alu.mult, op1=Alu.add)
    pf = pspool.tile([1, 1], f32, tag="pst", bufs=2, name="pf")
    nc.tensor.matmul(pf[:], comb[:], ones, start=True, stop=True)
    res = pool.tile([1, 1], f32, tag="res")
    nc.vector.tensor_copy(res[:], pf[:])
    nc.sync.dma_start(out_partial[:, :], res[:])

    ctx.close()


def build():
    import concourse.tile as tile
    from concourse import bacc, mybir

    nc = bacc.Bacc("TRN2", target_bir_lowering=False, debug=False,
                   enable_asserts=True, num_devices=_NCORES)
    projs = nc.dram_tensor("projs", [_N2, _D], mybir.dt.float32,
                           kind="ExternalInput").ap()
    consts = nc.dram_tensor("consts", [128, 129], mybir.dt.float32,
                            kind="ExternalInput").ap()
    out_partial = nc.dram_tensor("partial", [1, 1], mybir.dt.float32,
                                 kind="ExternalOutput").ap()
    with tile.TileContext(nc) as tc:
        _emit(tc, projs, consts, out_partial)
    nc.compile()
    return nc


_NC_CACHE = None


def _get_nc():
    global _NC_CACHE
    if _NC_CACHE is None:
        _NC_CACHE = build()
    return _NC_CACHE


def make_in_maps(proj_1, proj_2):
    z = np.concatenate([np.asarray(proj_1, dtype=np.float32),
                        np.asarray(proj_2, dtype=np.float32)], axis=0)
    consts = np.concatenate(
        [np.eye(128, dtype=np.float32),
         np.ones((128, 1), dtype=np.float32)], axis=1)
    return [{"projs": np.ascontiguousarray(np.roll(z, -_RPC * c, axis=0)),
             "consts": consts}
            for c in range(_NCORES)]


def kernel(proj_1, proj_2):
    from concourse import bass_utils

    nc = _get_nc()
    in_maps = make_in_maps(proj_1, proj_2)
    r = bass_utils.run_bass_kernel_spmd(nc, in_maps,
                                        core_ids=list(range(_NCORES)))
    total = sum(float(res["partial"][0, 0]) for res in r.results)
    return np.float32(total / _N2)


# revision 35
# speedup vs baseline: 1.4183x; 1.1317x over previous
"""NT-Xent / contrastive loss on 8 Trainium2 NeuronCores.

Reference computation (B=4096, D=512, temp=0.1):
    z   = l2norm(concat(proj_1, proj_2))          # [8192, 512]
    cos = (z @ z.T) / temp                        # [8192, 8192]
    pos[r]  = cos[r, (r + 4096) % 8192]
    lse[r]  = logsumexp(cos[r, :] with cos[r, r] masked out)
    loss    = mean(lse - pos)

Sharding: rows of the similarity matrix, 1024 per core.  Each core
receives the full stacked [8192, 512] input *rotated* by core*1024 rows,
which makes the program uniform across cores (SPMD): local rows 0..1023
are the core's rows, the self-diagonal sits at local column == row, and
the positive sits at local column == row + 4096.

Per core:
  1. SWDGE cast-DMA streams the 64 row-tiles in as bf16 (f32 read from
     HBM, bf16 write to SBUF) — no compute-engine cast pass.
  2. Row sumsq via one DVE scalar_tensor_tensor per tile (bf16 in);
     1/||row|| via fast-rsqrt (int magic) + 2 Newton steps on DVE.
  3. Normalization is folded into the PE transpose: instead of an
     identity, the transpose matmul's moving operand is diag(rn*S) so
     psT = z.T * rn * S drops out of the same 4 matmuls per tile.
     DVE evacuates PSUM straight to fp8e4 (S=64 keeps |zt| ~ 3).
  4. GEMM in fp8 with perf_mode=DoubleRow: contraction 512 done as two
     256-deep matmuls per 512-col chunk (2x PE throughput vs bf16).
     Columns grouped [1536,1536,1536,1536,1536,512] per row-block; one
     ScalarE Exp (scale=10/S^2) with accum_out per group gives the row
     sumexp.  Self/pos diagonals pulled out of raw PSUM with a
     multiply-by-identity reduce before the in-place Exp (self in
     group 0, pos in group 2 or 3, thanks to the input rotation).
  5. lse = ln(sumexp - exp(self*scale)); partial = sum(lse - scale*pos)
     over the core's 1024 rows, reduced to [1,1] via a ones-matmul.
Host adds the 8 partials and divides by 8192.

GEMM/transpose emission is interleaved so the Tile scheduler overlaps
the input stream (DMA/DVE/GpSimd) with the GEMM+exp pipeline (PE/ACT).
"""

import sys

import numpy as np

if "/opt/trn_rl_repo" not in sys.path:
    sys.path.insert(0, "/opt/trn_rl_repo")

_B = 4096
_D = 512
_N2 = 2 * _B            # 8192 rows of the similarity matrix
_NCORES = 8
_RPC = _N2 // _NCORES   # 1024 rows per core
_INV_TEMP = 10.0
_S = 64.0               # fp8 pre-scale on normalized rows
_SCL = _INV_TEMP / (_S * _S)   # logit scale applied at exp time

_NT = _N2 // 128        # 64 input row-tiles
_BATCHES = (8, 8, 8, 8, 8, 8, 8, 8)   # tiles per load/rsqrt batch
_NM = _RPC // 128       # 8 output row blocks per core
_NK = _D // 128         # 4 contraction chunks (2 DoubleRow pairs)
_NJ = _N2 // 512        # 16 column chunks of 512
# exp groups per row-block: small leading groups let the GEMM start after
# only 8 input tiles; 1536 amortizes ScalarE instruction overhead after.
_CGRP = (1024, 1024, 1536, 1536, 1536, 1536)

_MAGIC1 = 0x5F3759E0    # fast inverse sqrt magic + 1 (M - x == (M+1) + ~x)


def _emit(tc, projs, consts, out_partial):
    import concourse.bass as bass  # noqa: F401
    from concourse import mybir

    nc = tc.nc
    f32 = mybir.dt.float32
    bf16 = mybir.dt.bfloat16
    fp8 = mybir.dt.float8e4
    i32 = mybir.dt.int32
    Alu = mybir.AluOpType
    Act = mybir.ActivationFunctionType
    DR = mybir.MatmulPerfMode.DoubleRow

    from contextlib import ExitStack
    ctx = ExitStack()
    pool = ctx.enter_context(tc.tile_pool(name="work", bufs=1))
    pers = ctx.enter_context(tc.tile_pool(name="pers", bufs=1))
    pspool = ctx.enter_context(tc.tile_pool(name="psum", bufs=1, space="PSUM"))

    # ---- constants (host-provided: eye | ones column) so the identity
    # is never gated behind the gpsimd DMA-trigger queue ----
    cons = pers.tile([128, 129], f32, tag="cons")
    nc.sync.dma_start(cons[:], consts[:])
    ident = cons[:, 0:128]
    ones = cons[:, 128:129]
    identb = pers.tile([128, 128], bf16, tag="identb")
    nc.vector.tensor_copy(identb[:], ident)

    # ---- persistent buffers ----
    # zT, normalized*S, fp8: K-chunk k lives at columns [k*8192, (k+1)*8192).
    zt = pers.tile([128, _NK * _N2], fp8, tag="zt")
    zt3 = zt.rearrange("p (k c) -> p k c", k=_NK)
    # whole rotated input, cast to bf16 by the DMA engines (SWDGE)
    raw_all = pers.tile([128, _NT * _D], bf16, tag="raw")
    raw3 = raw_all.rearrange("p (t d) -> p t d", t=_NT)
    sp_all = pers.tile([128, 2 * _NM], f32, tag="sp")    # self diag | pos diag
    rs_all = pers.tile([128, _NM], f32, tag="rs")        # row sumexp per block
    se_all = pers.tile([128, _NM * len(_CGRP)], f32, tag="se")  # group sums
    se3 = se_all.rearrange("p (m g) -> p m g", m=_NM)

    # ---- phase 1: issue every cast-DMA upfront (gpsimd queue only) ----
    # First two batches go per-tile so the first tiles land ASAP; the rest
    # are batched (fewer instructions, same serial SWDGE stream).
    def emit_all_dmas():
        projs3 = projs.rearrange("(t p) d -> p t d", p=128)
        for t2 in range(_NT // 2):
            nc.gpsimd.dma_start(raw3[:, 2 * t2:2 * t2 + 2, :],
                                projs3[:, 2 * t2:2 * t2 + 2, :])

    # ---- per-batch compute chain (sumsq/rsqrt/diag/transpose/evac) ----
    def emit_load_group(g):
        t0 = sum(_BATCHES[:g])
        nb = _BATCHES[g]
        ssf = pool.tile([128, 16], f32, tag="ss", bufs=2, name=f"ss{g}")
        ss = ssf[:, 0:nb]
        for i in range(nb):
            t = t0 + i
            # row sumsq: split ScalarE (Square+accum) / DVE (STT) to
            # balance engine load; both land in ss[:, i].
            if 8 <= t < 16 or (t >= 16 and t % 16 in (0, 3, 6, 9, 12)):
                sq = pool.tile([128, _D], f32, tag="sqa", bufs=4,
                               name=f"sqa{t}")
                nc.scalar.activation(sq[:], raw3[:, t, :], Act.Square,
                                     bias=0.0, scale=1.0,
                                     accum_out=ss[:, i:i + 1])
            else:
                sq = pool.tile([128, _D], bf16, tag="sq", bufs=2,
                               name=f"sq{t}")
                nc.vector.scalar_tensor_tensor(
                    out=sq[:], in0=raw3[:, t, :], scalar=1.0,
                    in1=raw3[:, t, :],
                    op0=Alu.mult, op1=Alu.mult, accum_out=ss[:, i:i + 1])

        # rnorm = S/sqrt(max(ss, 1e-24)), fast-rsqrt + 2 Newton steps (DVE)
        sscf = pool.tile([128, 16], f32, tag="ssc", bufs=2, name=f"ssc{g}")
        ssc = sscf[:, 0:nb]
        nc.vector.tensor_scalar_max(ssc[:], ss[:], 1e-24)
        tif = pool.tile([128, 16], i32, tag="ti", bufs=2, name=f"ti{g}")
        ti = tif[:, 0:nb]
        nc.vector.tensor_scalar(
            out=ti[:], in0=ssc[:].bitcast(i32), scalar1=1, scalar2=-1,
            op0=Alu.logical_shift_right, op1=Alu.bitwise_xor)
        rnf = pool.tile([128, 16], f32, tag="rn", bufs=2, name=f"rn{g}")
        rn = rnf[:, 0:nb]
        nc.vector.tensor_scalar(
            out=rn[:].bitcast(i32), in0=ti[:], scalar1=_MAGIC1, scalar2=None,
            op0=Alu.add)
        ntf = pool.tile([128, 16], f32, tag="nt", bufs=2, name=f"nt{g}")
        nt = ntf[:, 0:nb]
        for _ in range(2):
            nc.vector.tensor_tensor(out=nt[:], in0=rn[:], in1=rn[:], op=Alu.mult)
            nc.vector.tensor_tensor(out=nt[:], in0=nt[:], in1=ssc[:], op=Alu.mult)
            nc.vector.tensor_scalar(out=nt[:], in0=nt[:], scalar1=-0.5,
                                    scalar2=1.5, op0=Alu.mult, op1=Alu.add)
            nc.vector.tensor_tensor(out=rn[:], in0=rn[:], in1=nt[:], op=Alu.mult)
        rnscf = pool.tile([128, 16], f32, tag="rnsc", bufs=2, name=f"rnsc{g}")
        rnsc = rnscf[:, 0:nb]
        nc.vector.tensor_scalar_mul(rnsc[:], rn[:], _S)

        for i in range(nb):
            t = t0 + i
            # diag(rn*S): identity column-scaled by per-partition scalar (DVE)
            diag = pool.tile([128, 128], bf16, tag="diag", bufs=8,
                             name=f"diag{t}")
            nc.vector.tensor_scalar_mul(diag[:], identb[:], rnsc[:, i:i + 1])
            # transpose + normalize in one: psT = raw.T @ diag(rn*S)
            psT = pspool.tile([128, _D], f32, tag="psT", bufs=2,
                              name=f"psT{t}")
            for d in range(_NK):
                nc.tensor.matmul(psT[:, d * 128:(d + 1) * 128],
                                 raw3[:, t, d * 128:(d + 1) * 128],
                                 diag[:], start=True, stop=True)
            # one strided evacuation: [128, 4, 128] f32 -> fp8
            dst = zt3[:, :, t * 128:(t + 1) * 128]
            src = psT[:].rearrange("p (k c) -> p k c", k=_NK)
            nc.vector.tensor_copy(dst, src)

    # ---- phase 2 helper: one (row-block m, col-group G) GEMM + exp ----
    def emit_gemm_group(m, G):
        width = _CGRP[G]
        col0 = sum(_CGRP[:G])
        psfull = pool_ps.tile([128, max(_CGRP)], f32, tag="ps", bufs=2,
                              name=f"ps{m}_{G}")
        ps = psfull[:, 0:width]
        # kk outer: consecutive matmuls share the stationary operand, so
        # LDWEIGHTS of the next chunk overlaps the running matmul cleanly.
        for kk in range(_NK // 2):
            for c in range(width // 512):
                j = col0 // 512 + c
                nc.tensor.matmul(
                    ps[:, c * 512:(c + 1) * 512],
                    zt3[:, 2 * kk:2 * kk + 2, m * 128:(m + 1) * 128],
                    zt3[:, 2 * kk:2 * kk + 2, j * 512:(j + 1) * 512],
                    start=(kk == 0), stop=(kk == _NK // 2 - 1),
                    perf_mode=DR)
        # diagonal extraction from raw PSUM (before in-place exp)
        selfoff = m * 128          # self diag lives in G0
        posoff = _B + m * 128      # pos diag in G2 (m<4) or G3 (m>=4)
        for col, off in ((m, selfoff), (_NM + m, posoff)):
            if col0 <= off and off + 128 <= col0 + width:
                junk = pool.tile([128, 128], f32, tag="junk", bufs=2,
                                 name=f"junk{m}_{G}")
                nc.vector.scalar_tensor_tensor(
                    out=junk[:], in0=ps[:, off - col0:off - col0 + 128],
                    scalar=1.0, in1=ident[:], op0=Alu.mult, op1=Alu.mult,
                    accum_out=sp_all[:, col:col + 1])
        nc.scalar.activation(ps[:], ps[:], Act.Exp, bias=0.0,
                             scale=_SCL, accum_out=se3[:, m, G:G + 1])
        if G == len(_CGRP) - 1:
            nc.vector.reduce_sum(out=rs_all[:, m:m + 1], in_=se3[:, m, :],
                                 axis=mybir.AxisListType.X)

    pool_ps = pspool  # alias: GEMM psum groups live in the same pool

    # ---- interleaved emission: stream tiles, fire GEMM groups when fed ----
    # group G of row-block m needs zt columns up to col0+width, i.e. input
    # tiles < ceil((col0+width)/128); tiles arrive in load-group batches of 8.
    emit_all_dmas()
    next_g = 0

    def tiles_ready():
        return sum(_BATCHES[:next_g])

    for G in range(len(_CGRP)):
        need = (sum(_CGRP[:G + 1]) + 127) // 128
        need = max(need, 8)  # lhs panel: tiles 0..7
        while tiles_ready() < need:
            emit_load_group(next_g)
            next_g += 1
        for m in range(_NM):
            emit_gemm_group(m, G)
    while next_g < len(_BATCHES):
        emit_load_group(next_g)
        next_g += 1

    # ---- phase 3: lse, loss, partial sum ----
    sx = pool.tile([128, _NM], f32, tag="sx")
    nc.scalar.activation(sx[:], sp_all[:, 0:_NM], Act.Exp, bias=0.0,
                         scale=_SCL)
    nc.vector.tensor_sub(rs_all[:], rs_all[:], sx[:])
    lse = pool.tile([128, _NM], f32, tag="lse")
    nc.scalar.activation(lse[:], rs_all[:], Act.Ln, bias=0.0, scale=1.0)
    loss = pool.tile([128, _NM], f32, tag="loss")
    nc.vector.scalar_tensor_tensor(
        out=loss[:], in0=sp_all[:, _NM:2 * _NM], scalar=-_SCL,
        in1=lse[:], op0=Alu.mult, op1=Alu.add)
    lossv = pool.tile([128, 1], f32, tag="lossv")
    nc.vector.reduce_sum(out=lossv[:], in_=loss[:], axis=mybir.AxisListType.X)
    pf = pspool.tile([1, 1], f32, tag="psT", bufs=2)
    nc.tensor.matmul(pf[:], lossv[:], ones, start=True, stop=True)
    res = pool.tile([1, 1], f32, tag="res")
    nc.vector.tensor_copy(res[:], pf[:])
    nc.sync.dma_start(out_partial[:, :], res[:])

    ctx.close()


def build():
    import concourse.tile as tile
    from concourse import bacc, mybir

    nc = bacc.Bacc("TRN2", target_bir_lowering=False, debug=False,
                   enable_asserts=True, num_devices=_NCORES)
    projs = nc.dram_tensor("projs", [_N2, _D], mybir.dt.float32,
                           kind="ExternalInput").ap()
    consts = nc.dram_tensor("consts", [128, 129], mybir.dt.float32,
                            kind="ExternalInput").ap()
    out_partial = nc.dram_tensor("partial", [1, 1], mybir.dt.float32,
                                 kind="ExternalOutput").ap()
    with tile.TileContext(nc) as tc:
        _emit(tc, projs, consts, out_partial)
    nc.compile()
    return nc


_NC_CACHE = None


def _get_nc():
    global _NC_CACHE
    if _NC_CACHE is None:
        _NC_CACHE = build()
    return _NC_CACHE


def make_in_maps(proj_1, proj_2):
    z = np.concatenate([np.asarray(proj_1, dtype=np.float32),
                        np.asarray(proj_2, dtype=np.float32)], axis=0)
    consts = np.concatenate(
        [np.eye(128, dtype=np.float32),
         np.ones((128, 1), dtype=np.float32)], axis=1)
    return [{"projs": np.ascontiguousarray(np.roll(z, -_RPC * c, axis=0)),
             "consts": consts}
            for c in range(_NCORES)]


def kernel(proj_1, proj_2):
    from concourse import bass_utils

    nc = _get_nc()
    in_maps = make_in_maps(proj_1, proj_2)
    r = bass_utils.run_bass_kernel_spmd(nc, in_maps,
                                        core_ids=list(range(_NCORES)))
    total = sum(float(res["partial"][0, 0]) for res in r.results)
    return np.float32(total / _N2)



# revision 36
# speedup vs baseline: 1.4405x; 1.0157x over previous
"""NT-Xent / contrastive loss on 8 Trainium2 NeuronCores.

Reference computation (B=4096, D=512, temp=0.1):
    z   = l2norm(concat(proj_1, proj_2))          # [8192, 512]
    cos = (z @ z.T) / temp                        # [8192, 8192]
    pos[r]  = cos[r, (r + 4096) % 8192]
    lse[r]  = logsumexp(cos[r, :] with cos[r, r] masked out)
    loss    = mean(lse - pos)

Sharding: rows of the similarity matrix, 1024 per core.  Each core
receives the full stacked [8192, 512] input *rotated* by core*1024 rows,
which makes the program uniform across cores (SPMD): local rows 0..1023
are the core's rows, the self-diagonal sits at local column == row, and
the positive sits at local column == row + 4096.

Per core:
  1. SWDGE cast-DMA streams the 64 row-tiles in as bf16 (f32 read from
     HBM, bf16 write to SBUF) — no compute-engine cast pass.
  2. Row sumsq via one DVE scalar_tensor_tensor per tile (bf16 in);
     1/||row|| via fast-rsqrt (int magic) + 2 Newton steps on DVE.
  3. Normalization is folded into the PE transpose: instead of an
     identity, the transpose matmul's moving operand is diag(rn*S) so
     psT = z.T * rn * S drops out of the same 4 matmuls per tile.
     DVE evacuates PSUM straight to fp8e4 (S=64 keeps |zt| ~ 3).
  4. GEMM in fp8 with perf_mode=DoubleRow: contraction 512 done as two
     256-deep matmuls per 512-col chunk (2x PE throughput vs bf16).
     Columns grouped [1536,1536,1536,1536,1536,512] per row-block; one
     ScalarE Exp (scale=10/S^2) with accum_out per group gives the row
     sumexp.  Self/pos diagonals pulled out of raw PSUM with a
     multiply-by-identity reduce before the in-place Exp (self in
     group 0, pos in group 2 or 3, thanks to the input rotation).
  5. lse = ln(sumexp - exp(self*scale)); partial = sum(lse - scale*pos)
     over the core's 1024 rows, reduced to [1,1] via a ones-matmul.
Host adds the 8 partials and divides by 8192.

GEMM/transpose emission is interleaved so the Tile scheduler overlaps
the input stream (DMA/DVE/GpSimd) with the GEMM+exp pipeline (PE/ACT).
"""

import sys

import numpy as np

if "/opt/trn_rl_repo" not in sys.path:
    sys.path.insert(0, "/opt/trn_rl_repo")

_B = 4096
_D = 512
_N2 = 2 * _B            # 8192 rows of the similarity matrix
_NCORES = 8
_RPC = _N2 // _NCORES   # 1024 rows per core
_INV_TEMP = 10.0
_S = 64.0               # fp8 pre-scale on normalized rows
_SCL = _INV_TEMP / (_S * _S)   # logit scale applied at exp time

_NT = _N2 // 128        # 64 input row-tiles
_BATCHES = (8, 8, 8, 8, 8, 8, 8, 8)   # tiles per load/rsqrt batch
_NM = _RPC // 128       # 8 output row blocks per core
_NK = _D // 128         # 4 contraction chunks (2 DoubleRow pairs)
_NJ = _N2 // 512        # 16 column chunks of 512
# exp groups per row-block: small leading groups let the GEMM start after
# only 8 input tiles; 1536 amortizes ScalarE instruction overhead after.
_CGRP = (1024, 1024, 1536, 1536, 1536, 1536)

_MAGIC1 = 0x5F3759E0    # fast inverse sqrt magic + 1 (M - x == (M+1) + ~x)


def _emit(tc, projs, consts, out_partial):
    import concourse.bass as bass  # noqa: F401
    from concourse import mybir

    nc = tc.nc
    f32 = mybir.dt.float32
    bf16 = mybir.dt.bfloat16
    fp8 = mybir.dt.float8e4
    i32 = mybir.dt.int32
    Alu = mybir.AluOpType
    Act = mybir.ActivationFunctionType
    DR = mybir.MatmulPerfMode.DoubleRow

    from contextlib import ExitStack
    ctx = ExitStack()
    pool = ctx.enter_context(tc.tile_pool(name="work", bufs=1))
    pers = ctx.enter_context(tc.tile_pool(name="pers", bufs=1))
    pspool = ctx.enter_context(tc.tile_pool(name="psum", bufs=1, space="PSUM"))

    # ---- constants (host-provided: eye | ones column) so the identity
    # is never gated behind the gpsimd DMA-trigger queue ----
    cons = pers.tile([128, 129], f32, tag="cons")
    nc.sync.dma_start(cons[:], consts[:])
    ident = cons[:, 0:128]
    ones = cons[:, 128:129]
    identb = pers.tile([128, 128], bf16, tag="identb")
    nc.vector.tensor_copy(identb[:], ident)

    # ---- persistent buffers ----
    # zT, normalized*S, fp8: K-chunk k lives at columns [k*8192, (k+1)*8192).
    zt = pers.tile([128, _NK * _N2], fp8, tag="zt")
    zt3 = zt.rearrange("p (k c) -> p k c", k=_NK)
    # whole rotated input, cast to bf16 by the DMA engines (SWDGE)
    raw_all = pers.tile([128, _NT * _D], bf16, tag="raw")
    raw3 = raw_all.rearrange("p (t d) -> p t d", t=_NT)
    sp_all = pers.tile([128, 2 * _NM], f32, tag="sp")    # self diag | pos diag
    rs_all = pers.tile([128, _NM], f32, tag="rs")        # row sumexp per block
    se_all = pers.tile([128, _NM * len(_CGRP)], f32, tag="se")  # group sums
    se3 = se_all.rearrange("p (m g) -> p m g", m=_NM)

    # ---- phase 1: issue every cast-DMA upfront (gpsimd queue only) ----
    # First two batches go per-tile so the first tiles land ASAP; the rest
    # are batched (fewer instructions, same serial SWDGE stream).
    def emit_all_dmas():
        for t in range(_NT):
            nc.gpsimd.dma_start(raw3[:, t, :],
                                projs[t * 128:(t + 1) * 128, :])

    # ---- per-batch compute chain (sumsq/rsqrt/diag/transpose/evac) ----
    def emit_load_group(g):
        t0 = sum(_BATCHES[:g])
        nb = _BATCHES[g]
        ssf = pool.tile([128, 16], f32, tag="ss", bufs=2, name=f"ss{g}")
        ss = ssf[:, 0:nb]
        for i in range(nb):
            t = t0 + i
            # row sumsq: split ScalarE (Square+accum) / DVE (STT) to
            # balance engine load; both land in ss[:, i].
            if 8 <= t < 16 or (t >= 16 and t % 16 in (0, 3, 6, 9, 12)):
                sq = pool.tile([128, _D], f32, tag="sqa", bufs=4,
                               name=f"sqa{t}")
                nc.scalar.activation(sq[:], raw3[:, t, :], Act.Square,
                                     bias=0.0, scale=1.0,
                                     accum_out=ss[:, i:i + 1])
            else:
                sq = pool.tile([128, _D], bf16, tag="sq", bufs=2,
                               name=f"sq{t}")
                nc.vector.scalar_tensor_tensor(
                    out=sq[:], in0=raw3[:, t, :], scalar=1.0,
                    in1=raw3[:, t, :],
                    op0=Alu.mult, op1=Alu.mult, accum_out=ss[:, i:i + 1])

        # rnorm = S/sqrt(max(ss, 1e-24)), fast-rsqrt + 2 Newton steps (DVE)
        sscf = pool.tile([128, 16], f32, tag="ssc", bufs=2, name=f"ssc{g}")
        ssc = sscf[:, 0:nb]
        nc.vector.tensor_scalar_max(ssc[:], ss[:], 1e-24)
        tif = pool.tile([128, 16], i32, tag="ti", bufs=2, name=f"ti{g}")
        ti = tif[:, 0:nb]
        nc.vector.tensor_scalar(
            out=ti[:], in0=ssc[:].bitcast(i32), scalar1=1, scalar2=-1,
            op0=Alu.logical_shift_right, op1=Alu.bitwise_xor)
        rnf = pool.tile([128, 16], f32, tag="rn", bufs=2, name=f"rn{g}")
        rn = rnf[:, 0:nb]
        nc.vector.tensor_scalar(
            out=rn[:].bitcast(i32), in0=ti[:], scalar1=_MAGIC1, scalar2=None,
            op0=Alu.add)
        ntf = pool.tile([128, 16], f32, tag="nt", bufs=2, name=f"nt{g}")
        nt = ntf[:, 0:nb]
        for _ in range(2):
            nc.vector.tensor_tensor(out=nt[:], in0=rn[:], in1=rn[:], op=Alu.mult)
            nc.vector.tensor_tensor(out=nt[:], in0=nt[:], in1=ssc[:], op=Alu.mult)
            nc.vector.tensor_scalar(out=nt[:], in0=nt[:], scalar1=-0.5,
                                    scalar2=1.5, op0=Alu.mult, op1=Alu.add)
            nc.vector.tensor_tensor(out=rn[:], in0=rn[:], in1=nt[:], op=Alu.mult)
        rnscf = pool.tile([128, 16], f32, tag="rnsc", bufs=2, name=f"rnsc{g}")
        rnsc = rnscf[:, 0:nb]
        nc.vector.tensor_scalar_mul(rnsc[:], rn[:], _S)

        for i in range(nb):
            t = t0 + i
            # diag(rn*S): identity column-scaled by per-partition scalar (DVE)
            diag = pool.tile([128, 128], bf16, tag="diag", bufs=8,
                             name=f"diag{t}")
            nc.vector.tensor_scalar_mul(diag[:], identb[:], rnsc[:, i:i + 1])
            # transpose + normalize in one: psT = raw.T @ diag(rn*S)
            psT = pspool.tile([128, _D], f32, tag="psT", bufs=2,
                              name=f"psT{t}")
            for d in range(_NK):
                nc.tensor.matmul(psT[:, d * 128:(d + 1) * 128],
                                 raw3[:, t, d * 128:(d + 1) * 128],
                                 diag[:], start=True, stop=True)
            # one strided evacuation: [128, 4, 128] f32 -> fp8
            dst = zt3[:, :, t * 128:(t + 1) * 128]
            src = psT[:].rearrange("p (k c) -> p k c", k=_NK)
            nc.vector.tensor_copy(dst, src)

    # ---- phase 2 helper: one (row-block m, col-group G) GEMM + exp ----
    def emit_gemm_group(m, G):
        width = _CGRP[G]
        col0 = sum(_CGRP[:G])
        psfull = pool_ps.tile([128, max(_CGRP)], f32, tag="ps", bufs=2,
                              name=f"ps{m}_{G}")
        ps = psfull[:, 0:width]
        # kk outer: consecutive matmuls share the stationary operand, so
        # LDWEIGHTS of the next chunk overlaps the running matmul cleanly.
        for kk in range(_NK // 2):
            for c in range(width // 512):
                j = col0 // 512 + c
                nc.tensor.matmul(
                    ps[:, c * 512:(c + 1) * 512],
                    zt3[:, 2 * kk:2 * kk + 2, m * 128:(m + 1) * 128],
                    zt3[:, 2 * kk:2 * kk + 2, j * 512:(j + 1) * 512],
                    start=(kk == 0), stop=(kk == _NK // 2 - 1),
                    perf_mode=DR)
        # diagonal extraction from raw PSUM (before in-place exp)
        selfoff = m * 128          # self diag lives in G0
        posoff = _B + m * 128      # pos diag in G2 (m<4) or G3 (m>=4)
        for col, off in ((m, selfoff), (_NM + m, posoff)):
            if col0 <= off and off + 128 <= col0 + width:
                junk = pool.tile([128, 128], f32, tag="junk", bufs=2,
                                 name=f"junk{m}_{G}")
                nc.vector.scalar_tensor_tensor(
                    out=junk[:], in0=ps[:, off - col0:off - col0 + 128],
                    scalar=1.0, in1=ident[:], op0=Alu.mult, op1=Alu.mult,
                    accum_out=sp_all[:, col:col + 1])
        nc.scalar.activation(ps[:], ps[:], Act.Exp, bias=0.0,
                             scale=_SCL, accum_out=se3[:, m, G:G + 1])
        if G == len(_CGRP) - 1:
            nc.vector.reduce_sum(out=rs_all[:, m:m + 1], in_=se3[:, m, :],
                                 axis=mybir.AxisListType.X)

    pool_ps = pspool  # alias: GEMM psum groups live in the same pool

    # ---- interleaved emission: stream tiles, fire GEMM groups when fed ----
    # group G of row-block m needs zt columns up to col0+width, i.e. input
    # tiles < ceil((col0+width)/128); tiles arrive in load-group batches of 8.
    emit_all_dmas()
    next_g = 0

    def tiles_ready():
        return sum(_BATCHES[:next_g])

    for G in range(len(_CGRP)):
        need = (sum(_CGRP[:G + 1]) + 127) // 128
        need = max(need, 8)  # lhs panel: tiles 0..7
        while tiles_ready() < need:
            emit_load_group(next_g)
            next_g += 1
        for m in range(_NM):
            emit_gemm_group(m, G)
    while next_g < len(_BATCHES):
        emit_load_group(next_g)
        next_g += 1

    # ---- phase 3: lse, loss, partial sum ----
    sx = pool.tile([128, _NM], f32, tag="sx")
    nc.scalar.activation(sx[:], sp_all[:, 0:_NM], Act.Exp, bias=0.0,
                         scale=_SCL)
    nc.vector.tensor_sub(rs_all[:], rs_all[:], sx[:])
    lse = pool.tile([128, _NM], f32, tag="lse")
    nc.scalar.activation(lse[:], rs_all[:], Act.Ln, bias=0.0, scale=1.0)
    loss = pool.tile([128, _NM], f32, tag="loss")
    nc.vector.scalar_tensor_tensor(
        out=loss[:], in0=sp_all[:, _NM:2 * _NM], scalar=-_SCL,
        in1=lse[:], op0=Alu.mult, op1=Alu.add)
    lossv = pool.tile([128, 1], f32, tag="lossv")
    nc.vector.reduce_sum(out=lossv[:], in_=loss[:], axis=mybir.AxisListType.X)
    pf = pspool.tile([1, 1], f32, tag="psT", bufs=2)
    nc.tensor.matmul(pf[:], lossv[:], ones, start=True, stop=True)
    res = pool.tile([1, 1], f32, tag="res")
    nc.vector.tensor_copy(res[:], pf[:])
    nc.sync.dma_start(out_partial[:, :], res[:])

    ctx.close()


def build():
    import concourse.tile as tile
    from concourse import bacc, mybir

    nc = bacc.Bacc("TRN2", target_bir_lowering=False, debug=False,
                   enable_asserts=True, num_devices=_NCORES)
    projs = nc.dram_tensor("projs", [_N2, _D], mybir.dt.float32,
                           kind="ExternalInput").ap()
    consts = nc.dram_tensor("consts", [128, 129], mybir.dt.float32,
                            kind="ExternalInput").ap()
    out_partial = nc.dram_tensor("partial", [1, 1], mybir.dt.float32,
                                 kind="ExternalOutput").ap()
    with tile.TileContext(nc) as tc:
        _emit(tc, projs, consts, out_partial)
    nc.compile()
    return nc


_NC_CACHE = None


def _get_nc():
    global _NC_CACHE
    if _NC_CACHE is None:
        _NC_CACHE = build()
    return _NC_CACHE


def make_in_maps(proj_1, proj_2):
    z = np.concatenate([np.asarray(proj_1, dtype=np.float32),
                        np.asarray(proj_2, dtype=np.float32)], axis=0)
    consts = np.concatenate(
        [np.eye(128, dtype=np.float32),
         np.ones((128, 1), dtype=np.float32)], axis=1)
    return [{"projs": np.ascontiguousarray(np.roll(z, -_RPC * c, axis=0)),
             "consts": consts}
            for c in range(_NCORES)]


def kernel(proj_1, proj_2):
    from concourse import bass_utils

    nc = _get_nc()
    in_maps = make_in_maps(proj_1, proj_2)
    r = bass_utils.run_bass_kernel_spmd(nc, in_maps,
                                        core_ids=list(range(_NCORES)))
    total = sum(float(res["partial"][0, 0]) for res in r.results)
    return np.float32(total / _N2)

